# revision 2
# baseline (speedup 1.0000x reference)
"""Trainium2 Bass kernel v2 for nn_ScoreGraphReconstructor (3-layer GATv2 + edge MLP).

Same sharding as v1 (nodes permuted into 8*WPC degree-balanced windows of 128;
each core owns the edges targeting its windows), but the edge phase is
rebalanced across engines:
  - m = xl[src] + xr[dst] is built ON THE PE: the xr one-hot recon matmuls and
    an identity matmul over the gathered gt accumulate into the same PSUM.
  - leaky-relu runs on ACT (Lrelu alpha=0.2) fused with the PSUM->SBUF copy.
  - the attention contraction is a packed bf16 DVE multiply against a
    pre-tiled att constant + a grouped reduce.
  - softmax has no max-subtraction (logits are small); exp on ACT.
  - ELU(x) = min(exp(x),1) + relu(x) - 1; the -1 is folded into the next
    layer's (otherwise zero) linear bias host-side, so ELU costs 2 ACT passes
    + 1 fused DVE op.
  - st/stT one-hots are host-precomputed and streamed from DRAM per window.
  - classifier: z1 = a[src] + b[dst] (transposed gather + PE recon/add),
    ELU via the same trick, z2/z3 feature-major matmuls.
"""

import sys

for _p in ("/opt/trn_rl_repo",):
    if _p not in sys.path:
        sys.path.insert(0, _p)

import numpy as np
from ml_dtypes import bfloat16 as np_bf16

import concourse.bass as bass
import concourse.bacc as bacc
import concourse.mybir as mybir
import concourse.tile as tile
from concourse.bass_utils import run_bass_kernel_spmd

F32 = mybir.dt.float32
BF16 = mybir.dt.bfloat16
I16 = mybir.dt.int16

NCORES = 8
H, C = 4, 64
HID = H * C  # 256
NCLS = 5
ACT = mybir.ActivationFunctionType
ALU = mybir.AluOpType

# ---- tunables (set from micro results) ----
SINGLE_PACKET = False     # dma_gather single_packet flag
EXB_ON_ACT = True         # materialize broadcast exp on ACT for packed DVE wx
LRELU_MODE = "act"        # "act" (Prelu w/ alpha), "dve" (copy+TS+max)
REDUCE_ON_GPSIMD = False  # att-contraction reduce on gpsimd instead of DVE
CLS_STT_ON_GPSIMD = False  # classifier ELU combine on gpsimd
DEBUG_DUMP = False        # dump intermediates as extra outputs


class Cfg:
    def __init__(self, n_nodes, n_edges, wpc, T, in_dim=256):
        self.n = n_nodes
        self.e = n_edges
        self.wpc = wpc
        self.T = T
        self.L = wpc * 128
        self.S = NCORES * self.L
        self.nwin = NCORES * self.wpc
        self.ew = T * 128
        self.ewg = (T - 1) * 128
        self.in_dim = in_dim


# ---------------------------------------------------------------- host prep


def _balance_windows(deg, nwin):
    import heapq

    n = len(deg)
    order = np.argsort(-deg, kind="stable")
    heap = [(0, w) for w in range(nwin)]
    heapq.heapify(heap)
    count = np.zeros(nwin, np.int64)
    slot_of = np.empty(n, np.int64)
    pos = np.zeros(nwin, np.int64)
    for node in order:
        while True:
            load, w = heapq.heappop(heap)
            if count[w] < 128:
                break
        slot_of[node] = w * 128 + pos[w]
        pos[w] += 1
        count[w] += 1
        if count[w] < 128:
            heapq.heappush(heap, (load + int(deg[node]), w))
    return slot_of


def _wrap_idx16(idx, rows=128):
    n = len(idx)
    assert n % 16 == 0
    blk = np.asarray(idx, np.int16).reshape(n // 16, 16).T
    return np.tile(blk, (rows // 16, 1))


def prepare_host(inputs, cfg):
    cn = {k: np.asarray(v) for k, v in inputs.items()}
    x = cn["x"].astype(np.float32)
    ei = cn["edge_index"].astype(np.int64)
    row, col = ei[0], ei[1]
    n, e = cfg.n, cfg.e
    L, S, wpc, T = cfg.L, cfg.S, cfg.wpc, cfg.T

    deg = np.bincount(col, minlength=n)
    slot_of = _balance_windows(deg, cfg.nwin)

    src_sl = slot_of[row]
    dst_sl = slot_of[col]
    orig_id = np.arange(e, dtype=np.int64)
    win = dst_sl // 128
    ordr = np.argsort(win, kind="stable")
    src_sl, dst_sl, win, orig_id = (
        src_sl[ordr], dst_sl[ordr], win[ordr], orig_id[ordr]
    )
    starts = np.searchsorted(win, np.arange(cfg.nwin))
    ends = np.searchsorted(win, np.arange(cfg.nwin), side="right")
    maxcnt = int((ends - starts).max())
    assert maxcnt <= cfg.ewg, f"window overflow: {maxcnt} > {cfg.ewg}; raise T"

    per_core = []
    out_maps = []
    ewg = cfg.ewg
    for c in range(NCORES):
        esrc = np.zeros((128, wpc, ewg // 16), np.int16)
        doff = np.full((128, wpc, T), 128, np.float32)
        poss, origs = [], []
        for wi in range(wpc):
            w = c * wpc + wi
            s0, s1 = starts[w], ends[w]
            cnt = s1 - s0
            srcw = np.zeros(ewg, np.int64)
            dofw = np.full(ewg, 128, np.int64)
            srcw[:cnt] = src_sl[s0:s1]
            dofw[:cnt] = dst_sl[s0:s1] - w * 128
            esrc[:, wi, :] = _wrap_idx16(srcw)
            doff[:, wi, : T - 1] = dofw.reshape(T - 1, 128).T
            doff[:, wi, T - 1] = np.arange(128)  # self-loop tile
            oid = orig_id[s0:s1]
            poss.append(wi * ewg + np.arange(cnt))
            origs.append(oid)
        # stT[n, wi, t, e] = (doff[e, wi, t] == n)   (recon rhs / z1 rhs)
        stT = (
            np.arange(128, dtype=np.int32)[:, None, None, None]
            == doff[None].transpose(0, 2, 3, 1)
        ).astype(np_bf16)
        # st[e, wi, t, n] = (doff[e, wi, t] == n)    (scatter lhsT)
        st = stT.transpose(3, 1, 2, 0).copy()
        per_core.append({"esrc": esrc, "stT": stT, "st": st})
        out_maps.append((np.concatenate(poss), np.concatenate(origs)))

    xp = np.zeros((S, cfg.in_dim), np.float32)
    xp[slot_of] = x
    nin = cfg.in_dim // 128

    # --- weights; ELU(-1) folded into downstream biases
    wmaps = {}
    for l in (1, 2, 3):
        kin = cfg.in_dim if l == 1 else HID
        Wl = cn[f"W{l}l"].astype(np.float32)
        Wr = cn[f"W{l}r"].astype(np.float32)
        bl = cn[f"b{l}l"].astype(np.float32).copy()
        br = cn[f"b{l}r"].astype(np.float32).copy()
        att = cn[f"att{l}"].astype(np.float32)
        wmaps[f"Wl{l}"] = Wl.reshape(kin // 128, 128, HID).astype(np_bf16)
        wmaps[f"Wr{l}"] = Wr.reshape(kin // 128, 128, HID).astype(np_bf16)
        wmaps[f"bl{l}"] = bl.reshape(1, HID)
        wmaps[f"br{l}"] = br.reshape(1, HID)
        # attbig: [128, T*HID] tiled copy of att for packed DVE multiply
        wmaps[f"attbig{l}"] = np.tile(
            att.reshape(1, HID), (128, T)
        ).astype(np_bf16)
        wmaps[f"biasb{l}"] = np.tile(
            cn[f"bias{l}"].astype(np.float32).reshape(1, HID), (128, 1)
        )
    Wc1 = cn["Wc1"].astype(np.float32)
    Wc2 = cn["Wc2"].astype(np.float32)
    Wc3 = cn["Wc3"].astype(np.float32)
    bc1 = cn["bc1"].astype(np.float32)
    bc2f = cn["bc2"].astype(np.float32)
    bc3f = cn["bc3"].astype(np.float32)
    wmaps["Wc1t"] = Wc1[:HID].reshape(2, 128, HID).astype(np_bf16)
    wmaps["Wc1b"] = Wc1[HID:].reshape(2, 128, HID).astype(np_bf16)
    wmaps["bc1"] = bc1.reshape(1, HID)
    wmaps["Wc2"] = Wc2.reshape(2, 128, HID // 2).astype(np_bf16)
    wmaps["Wc3"] = Wc3.astype(np_bf16)  # [128, 5]
    wmaps["bc2c"] = bc2f.reshape(HID // 2, 1)
    wmaps["bc3c"] = bc3f.reshape(NCLS, 1)
    wmaps["ones"] = np.ones((1, 512), np.float32)
    wmaps["identb"] = np.eye(128, dtype=np.float32).astype(np_bf16)
    wmaps["identf"] = np.eye(128, dtype=np.float32)

    in_maps = []
    for c in range(NCORES):
        m = dict(wmaps)
        m.update(per_core[c])
        xc = xp[c * L : (c + 1) * L].T.copy()
        m["xfm"] = xc.reshape(nin, 128, L).astype(np_bf16)
        in_maps.append(m)

    flags = (
        tuple(bool(np.any(np.abs(wmaps[f"bl{l}"]) > 0)
                   or np.any(np.abs(wmaps[f"br{l}"]) > 0)) for l in (1, 2, 3)),
        tuple(bool(np.any(cn[f"bias{l}"])) for l in (1, 2, 3)),
        bool(np.any(bc1)),
    )
    meta = {"slot_of": slot_of, "cfg": cfg, "out_maps": out_maps, "flags": flags}
    return in_maps, meta


# ---------------------------------------------------------------- device build


def build_nc(cfg, flags=((False, True, True), (False,) * 3, False), debug=False):
    nc = bacc.Bacc("TRN2", target_bir_lowering=False, debug=debug,
                   num_devices=NCORES)
    has_nbias, has_gbias, has_bc1 = flags
    L, S, wpc, T, ew = cfg.L, cfg.S, cfg.wpc, cfg.T, cfg.ew
    ewg = cfg.ewg
    nin = cfg.in_dim // 128

    P = {}

    def pin(name, shape, dtype=F32):
        P[name] = nc.declare_dram_parameter(name, list(shape), dtype,
                                            isOutput=False)

    pin("xfm", (nin, 128, L), BF16)
    pin("esrc", (128, wpc, ewg // 16), I16)
    pin("stT", (128, wpc, T, 128), BF16)
    pin("st", (128, wpc, T, 128), BF16)
    for l in (1, 2, 3):
        nk = nin if l == 1 else 2
        pin(f"Wl{l}", (nk, 128, HID), BF16)
        pin(f"Wr{l}", (nk, 128, HID), BF16)
        pin(f"bl{l}", (1, HID))
        pin(f"br{l}", (1, HID))
        pin(f"attbig{l}", (128, T * HID), BF16)
        pin(f"biasb{l}", (128, HID))
    pin("Wc1t", (2, 128, HID), BF16)
    pin("Wc1b", (2, 128, HID), BF16)
    pin("bc1", (1, HID))
    pin("Wc2", (2, 128, HID // 2), BF16)
    pin("Wc3", (128, NCLS), BF16)
    pin("bc2c", (HID // 2, 1))
    pin("bc3c", (NCLS, 1))
    pin("ones", (1, 512))
    pin("identb", (128, 128), BF16)
    pin("identf", (128, 128))
    out_t = nc.declare_dram_parameter("out_t", [NCLS, wpc * ewg], F32,
                                      isOutput=True)

    rg = [list(range(NCORES))]

    with tile.TileContext(nc) as tc:
        with (
            tc.tile_pool(name="const", bufs=1) as cp,
            tc.tile_pool(name="dram", bufs=1, space="DRAM") as dp,
            tc.tile_pool(name="work", bufs=4) as wp,
            tc.tile_pool(name="gtp", bufs=3) as gtp,
            tc.tile_pool(name="stp", bufs=3) as stp,
            tc.tile_pool(name="epbig", bufs=2) as ep,
            tc.tile_pool(name="ep1", bufs=2) as ep1,
            tc.tile_pool(name="psm", bufs=2, space="PSUM") as psm,   # m psum (2 banks each)
            tc.tile_pool(name="psn", bufs=2, space="PSUM") as psn,   # node/transp
            tc.tile_pool(name="pso", bufs=2, space="PSUM") as pso,   # scatter out
        ):
            # ---------- constants
            def load_const(name, dtype=F32, chunked=False):
                src = P[name]
                shp = list(src.shape)
                if chunked:
                    t = cp.tile([shp[1], shp[0], shp[2]], dtype, tag=name,
                                name=name + "_sb")
                    for c in range(shp[0]):
                        nc.sync.dma_start(t[:, c, :], src[c])
                else:
                    t = cp.tile(shp, dtype, tag=name, name=name + "_sb")
                    nc.sync.dma_start(t[:], src[:])
                return t

            xfm = [cp.tile([128, L], BF16, tag=f"xfm{c}", name=f"xfm{c}")
                   for c in range(nin)]
            for c in range(nin):
                nc.sync.dma_start(xfm[c][:], P["xfm"][c])
            esrc_sb = load_const("esrc", dtype=I16)
            consts = {}
            for l in (1, 2, 3):
                for nm in (f"Wl{l}", f"Wr{l}"):
                    consts[nm] = load_const(nm, dtype=BF16, chunked=True)
                consts[f"attbig{l}"] = load_const(f"attbig{l}", dtype=BF16)
                if has_nbias[l - 1]:
                    for nm in (f"bl{l}", f"br{l}"):
                        consts[nm] = load_const(nm)
                if has_gbias[l - 1]:
                    consts[f"biasb{l}"] = load_const(f"biasb{l}")
            for nm in ("Wc1t", "Wc1b", "Wc2"):
                consts[nm] = load_const(nm, dtype=BF16, chunked=True)
            consts["Wc3"] = load_const("Wc3", dtype=BF16)
            if has_bc1:
                consts["bc1"] = load_const("bc1")
            for nm in ("bc2c", "bc3c", "ones"):
                consts[nm] = load_const(nm)
            identb = load_const("identb", dtype=BF16)
            identf = load_const("identf")
            ones = consts["ones"]

            xr_w = cp.tile([128, wpc, HID], BF16, tag="xr_w")
            hbuf = cp.tile([128, wpc, HID], F32, tag="hbuf")
            b_win = cp.tile([128, wpc, HID], BF16, tag="b_win")

            xl_in = {l: dp.tile([L, HID], BF16, tag=f"xl_in{l}",
                                name=f"xl_in{l}") for l in (1, 2, 3)}
            xl_full = {l: dp.tile([S, HID], BF16, tag=f"xl_full{l}",
                                  name=f"xl_full{l}", addr_space="Shared")
                       for l in (1, 2, 3)}
            a_in = dp.tile([L, HID], BF16, tag="a_in")
            a_full = dp.tile([S, HID], BF16, tag="a_full", addr_space="Shared")

            def transposes(w):
                """hbuf[:, w, :] (bf16) -> xfm chunks (bf16, transposed)."""
                ws = slice(w * 128, (w + 1) * 128)
                tp = psn.tile([128, 512], F32, tag="psm")
                for c in range(2):
                    nc.tensor.transpose(
                        tp[:, c * 256 : c * 256 + 128],
                        hbuf[:, w, c * 128 : (c + 1) * 128],
                        identf[:],
                    )
                for c in range(2):
                    nc.vector.tensor_copy(
                        xfm[c][:, ws], tp[:, c * 256 : c * 256 + 128]
                    )

            def node(l, w):
                """xl (-> DRAM) and xr (-> SBUF) transforms for layer l."""
                ws = slice(w * 128, (w + 1) * 128)
                nk = nin if l == 1 else 2
                emit_bias = has_nbias[l - 1]
                ps = psn.tile([128, 512], F32, tag="psm")
                for side, Wn, bn, off in (
                    ("l", f"Wl{l}", f"bl{l}", 0),
                    ("r", f"Wr{l}", f"br{l}", HID),
                ):
                    pz = ps[:, off : off + HID]
                    for c in range(nk):
                        nc.tensor.matmul(
                            pz, lhsT=xfm[c][:, ws], rhs=consts[Wn][:, c, :],
                            start=(c == 0),
                            stop=(not emit_bias and c == nk - 1),
                        )
                    if emit_bias:
                        nc.tensor.matmul(
                            pz, lhsT=ones[:1, 0:128], rhs=consts[bn][:1, :],
                            start=False, stop=True,
                        )
                xo = wp.tile([128, HID], BF16, tag="xo")
                nc.vector.tensor_copy(xo[:], ps[:, 0:HID])
                nc.sync.dma_start(xl_in[l][ws, :], xo[:])
                nc.vector.tensor_copy(xr_w[:, w, :], ps[:, HID:512])

            def node_ab(w):
                """a = h3 @ Wc1t (-> DRAM), b = h3 @ Wc1b + bc1 (-> SBUF)."""
                ws = slice(w * 128, (w + 1) * 128)
                ps = psn.tile([128, 512], F32, tag="psm")
                pa = ps[:, 0:HID]
                pb = ps[:, HID:512]
                for c in range(2):
                    nc.tensor.matmul(pa, lhsT=xfm[c][:, ws],
                                     rhs=consts["Wc1t"][:, c, :],
                                     start=(c == 0), stop=(c == 1))
                for c in range(2):
                    nc.tensor.matmul(pb, lhsT=xfm[c][:, ws],
                                     rhs=consts["Wc1b"][:, c, :],
                                     start=(c == 0),
                                     stop=(not has_bc1 and c == 1))
                if has_bc1:
                    nc.tensor.matmul(pb, lhsT=ones[:1, 0:128],
                                     rhs=consts["bc1"][:1, :],
                                     start=False, stop=True)
                xo = wp.tile([128, HID], BF16, tag="xo")
                nc.scalar.activation(xo[:], pa, ACT.Copy)
                nc.sync.dma_start(a_in[ws, :], xo[:])
                nc.scalar.activation(b_win[:, w, :], pb, ACT.Copy)

            def edge(l, w, dbg=None):
                """GATv2 edge phase for window w of layer l -> hbuf[:, w, :]."""
                ws = slice(w * 128, (w + 1) * 128)
                # gathered xl[src] (edge-major) + local self tile
                gt = gtp.tile([128, T, HID], BF16, tag="gt")
                nc.gpsimd.dma_gather(
                    out_ap=gt[:, 0 : T - 1, :],
                    in_ap=xl_full[l][:],
                    idxs_ap=esrc_sb[:, w, :],
                    num_idxs=ewg,
                    num_idxs_reg=ewg,
                    elem_size=HID,
                    single_packet=SINGLE_PACKET,
                )
                nc.sync.dma_start(gt[:, T - 1, :], xl_in[l][ws, :])
                # streamed one-hots for this window
                st_sb = stp.tile([128, T, 128], BF16, tag="st_sb")
                nc.sync.dma_start(st_sb[:], P["st"][:, w])
                stT_sb = stp.tile([128, T, 128], BF16, tag="stT_sb")
                nc.sync.dma_start(stT_sb[:], P["stT"][:, w])

                # m = xr[dst] + gt on the PE; leaky-relu -> lrm (bf16)
                lrm = ep.tile([128, T, HID], BF16, tag="lrm")
                if LRELU_MODE == "dve":
                    msb = ep.tile([128, T, HID], BF16, tag="msb")
                GRP = 3  # tiles per psum group (3*256 = 768 cols)
                for tp_ in range(0, T, GRP):
                    k = min(GRP, T - tp_)
                    ps = psm.tile([128, 768], F32, tag="psm_e")
                    # sequential accumulation groups per 256-col segment
                    # (interleaving groups across segments corrupts PSUM)
                    for i in range(k):
                        nc.tensor.matmul(
                            ps[:, i * HID : (i + 1) * HID],
                            lhsT=stT_sb[:, tp_ + i, :],
                            rhs=xr_w[:, w, :],
                            start=True, stop=False,
                        )
                        nc.tensor.matmul(
                            ps[:, i * HID : (i + 1) * HID],
                            lhsT=identb[:],
                            rhs=gt[:, tp_ + i, :],
                            start=False, stop=True,
                        )
                    if LRELU_MODE == "dve":
                        nc.scalar.activation(
                            msb[:, tp_ : tp_ + k, :], ps[:, : k * HID],
                            ACT.Copy,
                        )
                    else:
                        nc.scalar.activation(
                            lrm[:, tp_ : tp_ + k, :], ps[:, : k * HID],
                            ACT.Prelu, alpha=0.2,
                        )
                if LRELU_MODE == "dve":
                    # lrelu = max(m, 0.2m): TS (4x bf16) then packed TT max
                    sc = ep.tile([128, T, HID], BF16, tag="sc")
                    nc.vector.tensor_scalar(
                        out=sc[:].rearrange("p t f -> p (t f)"),
                        in0=msb[:].rearrange("p t f -> p (t f)"),
                        scalar1=0.2, scalar2=None, op0=ALU.mult,
                    )
                    nc.vector.tensor_tensor(
                        out=lrm[:].rearrange("p t f -> p (t f)"),
                        in0=msb[:].rearrange("p t f -> p (t f)"),
                        in1=sc[:].rearrange("p t f -> p (t f)"),
                        op=ALU.max,
                    )
                # logits: packed multiply against tiled att + grouped reduce
                prod = ep.tile([128, T, HID], BF16, tag="prod")
                nc.vector.tensor_tensor(
                    out=prod[:].rearrange("p t f -> p (t f)"),
                    in0=lrm[:].rearrange("p t f -> p (t f)"),
                    in1=consts[f"attbig{l}"][:],
                    op=ALU.mult,
                )
                lg = ep1.tile([128, T * H], F32, tag="lg")
                red_eng = nc.gpsimd if REDUCE_ON_GPSIMD else nc.vector
                red_eng.tensor_reduce(
                    out=lg[:],
                    in_=prod[:].rearrange("p t (g c) -> p (t g) c", c=C),
                    axis=mybir.AxisListType.X,
                    op=ALU.add,
                )
                # ex (edge-major [128, T, H]) + optional broadcast-expanded exb
                ext = ep1.tile([128, T, H], BF16, tag="ext")
                nc.scalar.activation(
                    ext[:].rearrange("p t h -> p (t h)"), lg[:], ACT.Exp
                )
                wx = ep.tile([128, T, HID], BF16, tag="wx")
                if EXB_ON_ACT:
                    exb = ep.tile([128, T, HID], BF16, tag="exb")
                    nc.scalar.activation(
                        exb[:].rearrange("p t (h c) -> p t h c", c=C),
                        ext[:].unsqueeze(3).to_broadcast([128, T, H, C]),
                        ACT.Copy,
                    )
                    nc.vector.tensor_tensor(
                        out=wx[:].rearrange("p t f -> p (t f)"),
                        in0=gt[:].rearrange("p t f -> p (t f)"),
                        in1=exb[:].rearrange("p t f -> p (t f)"),
                        op=ALU.mult,
                    )
                else:
                    nc.vector.tensor_tensor(
                        out=wx[:].rearrange("p t (h c) -> p t h c", c=C),
                        in0=gt[:].rearrange("p t (h c) -> p t h c", c=C),
                        in1=ext[:].unsqueeze(3).to_broadcast([128, T, H, C]),
                        op=ALU.mult,
                    )
                if dbg is not None:
                    nc.sync.dma_start(dbg["dbg_gt"][:], gt[:])
                    nc.sync.dma_start(dbg["dbg_lrm"][:], lrm[:])
                    nc.sync.dma_start(dbg["dbg_ext"][:], ext[:])
                    nc.sync.dma_start(dbg["dbg_wx"][:], wx[:])
                    nc.sync.dma_start(dbg["dbg_stT"][:], stT_sb[:])
                    nc.sync.dma_start(dbg["dbg_st"][:], st_sb[:])
                    xrd = wp.tile([128, HID], BF16, tag="xo")
                    nc.vector.tensor_copy(xrd[:], xr_w[:, w, :])
                    nc.sync.dma_start(dbg["dbg_xr"][:], xrd[:])
                # scatter: ops[n, 0:HID] = sum st.T @ wx ; ops[n, HID:+H] = den
                ops = pso.tile([128, HID + H], F32, tag="ops")
                for t in range(T):
                    nc.tensor.matmul(
                        ops[:, 0:HID], lhsT=st_sb[:, t, :], rhs=wx[:, t, :],
                        start=(t == 0), stop=(t == T - 1),
                    )
                for t in range(T):
                    nc.tensor.matmul(
                        ops[:, HID : HID + H], lhsT=st_sb[:, t, :],
                        rhs=ext[:, t, :],
                        start=(t == 0), stop=(t == T - 1),
                    )
                rc = ep1.tile([128, H], F32, tag="rc")
                nc.vector.reciprocal(rc[:], ops[:, HID : HID + H])
                hb = hbuf[:, w, :]
                nc.vector.tensor_tensor(
                    out=hb.rearrange("p (h c) -> p h c", c=C),
                    in0=ops[:, 0:HID].rearrange("p (h c) -> p h c", c=C),
                    in1=rc[:].unsqueeze(2).to_broadcast([128, H, C]),
                    op=ALU.mult,
                )
                if has_gbias[l - 1]:
                    nc.vector.tensor_tensor(
                        out=hb, in0=hb, in1=consts[f"biasb{l}"][:], op=ALU.add
                    )
                if l <= 2:
                    # ELU: h = (min(exp(h),1) - 1) + relu(h)
                    et = ep1.tile([128, HID], F32, tag="et")
                    nc.scalar.activation(et[:], hb, ACT.Exp)
                    rt = ep1.tile([128, HID], F32, tag="rt")
                    nc.scalar.activation(rt[:], hb, ACT.Relu)
                    qt = ep1.tile([128, HID], F32, tag="qt")
                    nc.vector.tensor_scalar(
                        out=qt[:], in0=et[:], scalar1=1.0, scalar2=-1.0,
                        op0=ALU.min, op1=ALU.add,
                    )
                    nc.vector.tensor_tensor(
                        out=hb, in0=qt[:], in1=rt[:], op=ALU.add,
                    )

            def cls(w):
                """Edge classifier over window w's edge slots (feature-major)."""
                agT = gtp.tile([128, HID // 128, ewg], BF16, tag="agT")
                nc.gpsimd.dma_gather(
                    out_ap=agT[:],
                    in_ap=a_full[:],
                    idxs_ap=esrc_sb[:, w, :],
                    num_idxs=ewg,
                    num_idxs_reg=ewg,
                    elem_size=HID,
                    transpose=True,
                    single_packet=SINGLE_PACKET,
                )
                stT_sb = stp.tile([128, T, 128], BF16, tag="stT_sb")
                nc.sync.dma_start(stT_sb[:], P["stT"][:, w])
                ECH = 512
                NCH = ewg // ECH
                cls_stt = nc.gpsimd if CLS_STT_ON_GPSIMD else nc.vector
                z1 = ep.tile([128, 2, ewg], BF16, tag="prod")
                e1 = ep.tile([128, 2, ewg], F32, tag="e1f")
                r1 = ep.tile([128, 2, ewg], BF16, tag="msb")
                for mh in range(2):
                    for ch in range(NCH):
                        ps1f = psm.tile([128, 768], F32, tag="psm_e")
                        ps1 = ps1f[:, 0:512]
                        for i in range(ECH // 128):
                            t = ch * (ECH // 128) + i
                            seg = slice(i * 128, (i + 1) * 128)
                            nc.tensor.matmul(
                                ps1[:, seg],
                                lhsT=b_win[:, w, mh * 128 : (mh + 1) * 128],
                                rhs=stT_sb[:, t, :],
                                start=True, stop=False,
                            )
                            nc.tensor.matmul(
                                ps1[:, seg],
                                lhsT=identb[:],
                                rhs=agT[:, mh, ch * ECH + i * 128 :
                                        ch * ECH + (i + 1) * 128],
                                start=False, stop=True,
                            )
                        es = slice(ch * ECH, (ch + 1) * ECH)
                        # ELU+1: z1 = min(exp(ps),1) + relu(ps); -1 folded
                        # into bc2.
                        nc.scalar.activation(e1[:, mh, es], ps1[:], ACT.Exp)
                        nc.vector.tensor_scalar(
                            out=r1[:, mh, es], in0=ps1[:], scalar1=0.0,
                            scalar2=None, op0=ALU.max,
                        )
                # z1 = (min(exp,1) - 1) + relu
                nc.vector.tensor_scalar(
                    out=e1[:].rearrange("p m e -> p (m e)"),
                    in0=e1[:].rearrange("p m e -> p (m e)"),
                    scalar1=1.0, scalar2=-1.0, op0=ALU.min, op1=ALU.add,
                )
                cls_stt.tensor_tensor(
                    out=z1[:].rearrange("p m e -> p (m e)"),
                    in0=e1[:].rearrange("p m e -> p (m e)"),
                    in1=r1[:].rearrange("p m e -> p (m e)"),
                    op=ALU.add,
                )
                z2 = ep.tile([128, ewg], BF16, tag="wx")
                e2 = ep.tile([128, ewg], F32, tag="e2f")
                r2 = ep.tile([128, ewg], BF16, tag="exb")
                for ch in range(NCH):
                    es = slice(ch * ECH, (ch + 1) * ECH)
                    ps2f = psm.tile([128, 768], F32, tag="psm_e")
                    ps2 = ps2f[:, 0:512]
                    for mh in range(2):
                        nc.tensor.matmul(
                            ps2[:], lhsT=consts["Wc2"][:, mh, :],
                            rhs=z1[:, mh, es],
                            start=(mh == 0), stop=(mh == 1),
                        )
                    # ELU+1 with per-partition bias bc2 inside; -1 folded
                    # into bc3.
                    nc.scalar.activation(
                        e2[:, es], ps2[:], ACT.Exp, bias=consts["bc2c"][:, 0:1]
                    )
                    nc.vector.tensor_scalar(
                        out=r2[:, es], in0=ps2[:],
                        scalar1=consts["bc2c"][:, 0:1], scalar2=0.0,
                        op0=ALU.add, op1=ALU.max,
                    )
                nc.vector.tensor_scalar(
                    out=e2[:], in0=e2[:], scalar1=1.0, scalar2=-1.0,
                    op0=ALU.min, op1=ALU.add,
                )
                cls_stt.tensor_tensor(
                    out=z2[:], in0=e2[:], in1=r2[:], op=ALU.add,
                )
                zo = ep1.tile([NCLS, ewg], F32, tag="zo")
                for ch in range(NCH):
                    es = slice(ch * ECH, (ch + 1) * ECH)
                    ps3f = psm.tile([128, 768], F32, tag="psm_e")
                    ps3 = ps3f[:, 0:512]
                    nc.tensor.matmul(
                        ps3[:NCLS, :], lhsT=consts["Wc3"][:], rhs=z2[:, es],
                        start=True, stop=True,
                    )
                    nc.scalar.activation(
                        zo[:, es], ps3[:NCLS, :], ACT.Identity,
                        bias=consts["bc3c"][:, 0:1],
                    )
                nc.sync.dma_start(out_t[:, w * ewg : (w + 1) * ewg], zo[:])

            # ================= schedule
            def ag(src_t, dst_t):
                nc.gpsimd.collective_compute(
                    "AllGather", ALU.bypass, replica_groups=rg,
                    ins=[src_t[:].opt()], outs=[dst_t[:].opt()],
                )

            dbg = {}
            if DEBUG_DUMP:
                for nm, shp, dt in (
                    ("dbg_xl1", (L, HID), BF16),
                    ("dbg_h1", (128, wpc, HID), BF16),
                    ("dbg_xf0", (128, L), BF16),
                    ("dbg_a", (L, HID), BF16),
                    ("dbg_gt", (128, T, HID), BF16),
                    ("dbg_lrm", (128, T, HID), BF16),
                    ("dbg_ext", (128, T, H), BF16),
                    ("dbg_wx", (128, T, HID), BF16),
                    ("dbg_stT", (128, T, 128), BF16),
                    ("dbg_st", (128, T, 128), BF16),
                    ("dbg_xr", (128, HID), BF16),
                ):
                    dbg[nm] = nc.declare_dram_parameter(
                        nm, list(shp), dt, isOutput=True
                    )

            for w in range(wpc):
                node(1, w)
            ag(xl_in[1], xl_full[1])
            if DEBUG_DUMP:
                nc.sync.dma_start(dbg["dbg_xl1"][:], xl_in[1][:])
            for l in (1, 2, 3):
                for w in range(wpc):
                    edge(l, w,
                         dbg=dbg if (DEBUG_DUMP and l == 1 and w == 0) else None)
                    transposes(w)
                    if l < 3:
                        node(l + 1, w)
                    else:
                        node_ab(w)
                if DEBUG_DUMP and l == 1:
                    nc.sync.dma_start(dbg["dbg_h1"][:], hbuf[:])
                    nc.sync.dma_start(dbg["dbg_xf0"][:], xfm[0][:])
                if l < 3:
                    ag(xl_in[l + 1], xl_full[l + 1])
                else:
                    ag(a_in, a_full)
            if DEBUG_DUMP:
                nc.sync.dma_start(dbg["dbg_a"][:], a_in[:])
            for w in range(wpc):
                cls(w)

    nc.compile()
    return nc


# ---------------------------------------------------------------- entry point

_CACHE = {}


def run(inputs, cfg, **kw):
    in_maps, meta = prepare_host(inputs, cfg)
    key = (cfg.n, cfg.e, cfg.wpc, cfg.T, cfg.in_dim, meta["flags"])
    if key not in _CACHE:
        _CACHE[key] = build_nc(cfg, flags=meta["flags"])
    nc = _CACHE[key]
    res = run_bass_kernel_spmd(nc, in_maps, list(range(NCORES)), **kw)
    out = np.zeros((cfg.e, NCLS), np.float32)
    for c in range(NCORES):
        o = np.asarray(res.results[c]["out_t"], np.float32)
        pos, orig = meta["out_maps"][c]
        out[orig] = o[:, pos].T
    return out, res


def kernel(**inputs) -> np.ndarray:
    n = inputs["x"].shape[0]
    e = inputs["edge_index"].shape[1]
    wpc = -(-n // (NCORES * 128))
    cfg = Cfg(n, e, wpc=wpc, T=9, in_dim=inputs["x"].shape[1])
    while True:
        try:
            out, _ = run(inputs, cfg)
            return out
        except AssertionError as ex:
            if "window overflow" in str(ex) and cfg.T < 16:
                cfg = Cfg(n, e, wpc=wpc, T=cfg.T + 1, in_dim=inputs["x"].shape[1])
                continue
            raise
